# revision 17
# baseline (speedup 1.0000x reference)
import sys, os
sys.path.insert(0, "/opt/trn_rl_repo")
import numpy as np

import concourse.bass as bass
import concourse.bacc as bacc
import concourse.tile as tile
from concourse import mybir
from concourse.bass_utils import run_bass_kernel_spmd

B, S, D = 1024, 256, 16
NB = 2
NCORES = 8
BS = B // NCORES          # 128 batch rows per core
NG = BS // 8              # 16 groups of 8 batch rows
EPS = 1e-5
F32 = mybir.dt.float32
BF16 = mybir.dt.bfloat16

_CACHE = {}


def _make_pe():
    pos = np.arange(300)[:, None].astype(np.float32)
    div = np.exp(np.arange(0, D, 2).astype(np.float32) * (-np.log(10000.0) / D))
    pe = np.zeros((300, D), dtype=np.float32)
    pe[:, 0::2] = np.sin(pos * div)
    pe[:, 1::2] = np.cos(pos * div)
    return pe[:S]


def _build_program():
    nc = bacc.Bacc()
    NBLOB = 10 * 128 + 128 + 64 + 64 + 10 + NG * S
    blob_d = nc.dram_tensor("blob", [128, NBLOB], F32, kind="ExternalInput")
    out_d = nc.dram_tensor("out", [128, NG * S], F32, kind="ExternalOutput")

    with tile.TileContext(nc) as tc:
        from contextlib import ExitStack
        ctx = ExitStack()
        consts = ctx.enter_context(tc.tile_pool(name="consts", bufs=1))
        state = ctx.enter_context(tc.tile_pool(name="state", bufs=1))
        vaugp = ctx.enter_context(tc.tile_pool(name="vaug", bufs=3))
        expp = ctx.enter_context(tc.tile_pool(name="expp", bufs=3))
        atsb = ctx.enter_context(tc.tile_pool(name="atsb", bufs=2))
        sml = ctx.enter_context(tc.tile_pool(name="sml", bufs=4))
        psA = ctx.enter_context(tc.tile_pool(name="psA", bufs=2, space="PSUM"))
        psQ = ctx.enter_context(tc.tile_pool(name="psQ", bufs=2, space="PSUM"))
        psR = ctx.enter_context(tc.tile_pool(name="psR", bufs=1, space="PSUM"))
        psD = ctx.enter_context(tc.tile_pool(name="psD", bufs=1, space="PSUM"))

        # ---- one blob DMA for every input ----
        blob = consts.tile([128, NBLOB], F32, tag="blob")
        nc.gpsimd.dma_start(out=blob, in_=blob_d[:, :])
        pe_touch_pending = True
        off = 0
        wt = {}
        for nm in ("wq", "wk", "wv", "w1", "w2"):
            for blk in range(NB):
                wt[(nm, blk)] = blob[:, off:off + 128]
                off += 128
        iden = blob[:, off:off + 128]; off += 128
        pat = blob[:, off:off + 64]; off += 64
        rbp = blob[:, off:off + 64]; off += 64
        bt = {}
        for nm in ("cbq", "cbk", "cbv", "cb1", "cb2"):
            for blk in range(NB):
                bt[(nm, blk)] = blob[:, off:off + 1]
                off += 1
        xblob = blob[:, off:off + NG * S]
        idenb = consts.tile([128, 128], BF16, tag="idenb")
        nc.vector.tensor_copy(out=idenb, in_=iden)
        patb = consts.tile([128, 64], BF16, tag="patb")
        nc.vector.tensor_copy(out=patb, in_=pat)
        rbpb = consts.tile([128, 64], BF16, tag="rbpb")
        nc.vector.tensor_copy(out=rbpb, in_=rbp)
        zeros_c = consts.tile([128, 1], F32, tag="zeros")
        nc.vector.memset(zeros_c, 0.0)
        inv128 = consts.tile([128, 1], BF16, tag="inv128")
        nc.vector.memset(inv128, 1.0 / 128.0)
        ones_r = consts.tile([1, 128], F32, tag="ones_r")
        nc.vector.memset(ones_r, 1.0)
        eps_t = consts.tile([1, 1], F32, tag="eps")
        nc.vector.memset(eps_t, EPS)

        d_tile = psD.tile([1, 1], F32, tag="d")

        def pe_touch(ap):
            nc.tensor.matmul(d_tile, ap, ap, start=True, stop=True)

        # ---- state tiles ----
        A = state.tile([128, NG, S], F32, tag="A")   # x / z
        Y = state.tile([128, NG, S], F32, tag="Y")   # residual accum
        QK = state.tile([128, 2, NG, S], BF16, tag="QK")
        V = state.tile([128, NG, S], BF16, tag="V")
        QKS = state.tile([112, 2, NG, S], BF16, tag="QKS")
        H = state.tile([128, NG, S], F32, tag="H")

        nc.vector.tensor_copy(out=A, in_=xblob.rearrange('p (g s) -> p g s', g=NG))
        pe_touch(blob[0:1, 0:1])
        pe_touch(A[0:1, 0, 0:1])

        def layernorm(src, dst):
            stats = sml.tile([128, NG, 6], F32, tag="stats")
            for g in range(NG):
                nc.vector.bn_stats(out=stats[:, g, :], in_=src[:, g, :])
            mv = sml.tile([128, 2], F32, tag="mv")
            nc.vector.bn_aggr(out=mv, in_=stats)
            # build [mean, Ex2] per partition
            ms = sml.tile([128, 2], F32, tag="ms")
            nc.vector.tensor_mul(out=ms[:, 1:2], in0=mv[:, 0:1], in1=mv[:, 0:1])
            nc.vector.tensor_add(out=ms[:, 1:2], in0=ms[:, 1:2], in1=mv[:, 1:2])
            nc.vector.tensor_copy(out=ms[:, 0:1], in_=mv[:, 0:1])
            msb = sml.tile([128, 2], BF16, tag="msb")
            nc.vector.tensor_copy(out=msb, in_=ms)
            pstat = psQ.tile([1, 2], F32, tag="quad")
            nc.tensor.matmul(pstat, inv128, msb, start=True, stop=True)
            gm = sml.tile([1, 4], F32, tag="gm")
            # gm[0]=mean, gm[1]=Ex2 -> var, then rstd
            nc.vector.tensor_copy(out=gm[:, 0:2], in_=pstat)
            nc.vector.tensor_mul(out=gm[:, 2:3], in0=gm[:, 0:1], in1=gm[:, 0:1])
            nc.vector.tensor_tensor(out=gm[:, 1:2], in0=gm[:, 1:2], in1=gm[:, 2:3],
                                    op=mybir.AluOpType.subtract)
            nc.scalar.activation(out=gm[:, 1:2], in_=gm[:, 1:2],
                                 func=mybir.ActivationFunctionType.Sqrt,
                                 bias=eps_t, scale=1.0)
            nc.vector.reciprocal(out=gm[:, 1:2], in_=gm[:, 1:2])
            gm2 = sml.tile([1, 2], F32, tag="gm2")
            nc.vector.tensor_copy(out=gm2, in_=gm[:, 0:2])
            pe_touch(gm2[0:1, 0:1])
            pbc = psQ.tile([128, 2], F32, tag="quad")
            nc.tensor.matmul(pbc, ones_r, gm2, start=True, stop=True)
            sc = sml.tile([128, 2], F32, tag="sc")
            nc.vector.tensor_copy(out=sc, in_=pbc)
            for g in range(NG):
                nc.vector.tensor_scalar(out=dst[:, g, :], in0=src[:, g, :],
                                        scalar1=sc[:, 0:1], scalar2=sc[:, 1:2],
                                        op0=mybir.AluOpType.subtract,
                                        op1=mybir.AluOpType.mult)
            pe_touch(dst[0:1, 0, 0:1])

        for blk in range(NB):
            # ---- projections q,k,v ----
            for nm, bnm, dsti in (("wq", "cbq", 0), ("wk", "cbk", 1), ("wv", "cbv", 2)):
                for gg in range(NG // 2):
                    ps = psA.tile([128, 2, S], F32, tag="big")
                    nc.tensor.matmul(ps, wt[(nm, blk)], A[:, 2 * gg:2 * gg + 2, :],
                                     start=True, stop=True)
                    dst = V[:, 2 * gg:2 * gg + 2, :] if dsti == 2 else \
                        QK[:, dsti, 2 * gg:2 * gg + 2, :]
                    nc.vector.tensor_scalar_add(out=dst, in0=ps,
                                                scalar1=bt[(bnm, blk)])
            # parity-shifted copy for odd-b score slicing (single DMA)
            nc.gpsimd.dma_start(out=QKS, in_=QK[16:128])
            pe_touch(QKS[0:1, 0, 0, 0:1])

            for g in range(NG):
                # ---- v transpose -> v_aug [128 tok, 8, 17] per chunk ----
                vaug = []
                for c in range(2):
                    pst = psA.tile([128, 128], BF16, tag="big")
                    nc.tensor.transpose(pst, V[:, g, 128 * c:128 * (c + 1)], idenb)
                    va = vaugp.tile([128, 8, 32], BF16, tag="va")
                    nc.vector.tensor_copy(
                        out=va[:, :, 0:16],
                        in_=pst.rearrange("p (b d) -> p b d", b=8))
                    nc.vector.memset(va[:, :, 16], 1.0)
                    nc.vector.memset(va[:, :, 17:32], 1.0)
                    vaug.append(va)
                pe_touch(vaug[1][0:1, 0, 0:1])
                # ---- scores + exp, per pair of b ----
                expt = []
                for bp in range(4):
                    pss = psA.tile([128, 2, 2, S], F32, tag="big")
                    for bl in range(2):
                        b = 2 * bp + bl
                        base = 16 * b - 16 * bl
                        src_t = QK if bl == 0 else QKS
                        for c in range(2):
                            nc.tensor.matmul(
                                pss[:, bl, c, :],
                                src_t[base:base + 16, 1, g, 128 * c:128 * (c + 1)],
                                src_t[base:base + 16, 0, g, :],
                                start=True, stop=True,
                                tile_position=(base, 0))
                    et = expp.tile([128, 2, 2, S], BF16, tag="exp")
                    nc.scalar.activation(out=et, in_=pss,
                                         func=mybir.ActivationFunctionType.Exp,
                                         scale=0.25)
                    expt.append(et)
                # ---- attention: 2 quads, col-tiled ----
                asb = atsb.tile([128, 2, S], BF16, tag="asb")
                asbf = atsb.tile([128, 2, S], F32, tag="asbf")
                for qd in range(2):
                    pa = psQ.tile([128, S], F32, tag="quad")
                    for j in range(4):
                        b = 4 * qd + j
                        et = expt[b // 2]
                        for c in range(2):
                            nc.tensor.matmul(
                                pa[32 * j:32 * j + 32, :],
                                vaug[c][:, b % 8, :],
                                et[:, b % 2, c, :],
                                start=(c == 0), stop=(c == 1),
                                tile_position=(0, 32 * j))
                    nc.vector.tensor_copy(out=asb[:, qd, :], in_=pa)
                    nc.vector.tensor_copy(out=asbf[:, qd, :], in_=pa)
                # reciprocal (full tile; only denom rows are consumed by rbp)
                rcp = sml.tile([128, 2, S], F32, tag="rcp")
                nc.vector.reciprocal_approx_fast(out=rcp, in_=asbf)
                rcpb = sml.tile([128, 2, S], BF16, tag="rcpb")
                nc.vector.tensor_copy(out=rcpb, in_=rcp)
                # ---- repack + recip broadcast via pattern matmuls ----
                prr = psR.tile([128, 2, S], F32, tag="pr")
                pr = prr[:, 0, :]
                prb = prr[:, 1, :]
                for qd in range(2):
                    nc.tensor.matmul(pr[64 * qd:64 * (qd + 1), :], patb,
                                     asb[:, qd, :], start=True, stop=True,
                                     tile_position=(0, 64 * qd))
                    nc.tensor.matmul(prb[64 * qd:64 * (qd + 1), :], rbpb,
                                     rcpb[:, qd, :], start=True, stop=True,
                                     tile_position=(0, 64 * qd))
                rbs = sml.tile([128, S], F32, tag="rbs")
                nc.vector.tensor_copy(out=rbs, in_=prb)
                an = sml.tile([128, S], F32, tag="an")
                nc.vector.tensor_mul(out=an, in0=pr, in1=rbs)
                nc.vector.tensor_add(out=Y[:, g, :], in0=an, in1=A[:, g, :])

            layernorm(Y, A)

            # ---- FFN ----
            for gg in range(NG // 2):
                sl = slice(2 * gg, 2 * gg + 2)
                ps = psA.tile([128, 2, S], F32, tag="big")
                nc.tensor.matmul(ps, wt[("w1", blk)], A[:, sl, :], start=True, stop=True)
                nc.vector.tensor_scalar(out=H[:, sl, :], in0=ps,
                                        scalar1=bt[("cb1", blk)], scalar2=zeros_c,
                                        op0=mybir.AluOpType.add,
                                        op1=mybir.AluOpType.max)
                ps2 = psA.tile([128, 2, S], F32, tag="big")
                nc.tensor.matmul(ps2, wt[("w2", blk)], H[:, sl, :], start=True, stop=True)
                ff = sml.tile([128, 2, S], F32, tag="ff")
                nc.vector.tensor_scalar_add(out=ff, in0=ps2, scalar1=bt[("cb2", blk)])
                nc.vector.tensor_add(out=Y[:, sl, :], in0=ff, in1=A[:, sl, :])

            layernorm(Y, A)

        nc.gpsimd.dma_start(out=out_d[:, :].rearrange('p (g s) -> p g s', g=NG), in_=A)
        ctx.close()
    nc.finalize()
    return nc


def _host_prep(tokens, embed, Wq, bq, Wk, bk, Wv, bv, W1, b1, W2, b2):
    tokens = np.asarray(tokens)
    x0 = np.asarray(embed, np.float32)[tokens] + _make_pe()[None, :, :]  # [B,S,D]
    pat = np.zeros((128, 64), np.float32)
    rbq = np.zeros((128, 64), np.float32)
    for c in range(4):
        for d in range(16):
            pat[32 * c + d, 16 * c + d] = 1.0
            rbq[32 * c + 16, 16 * c + d] = 1.0
    Ws = {"wq": Wq, "wk": Wk, "wv": Wv, "w1": W1, "w2": W2}
    Bs = {"cbq": bq, "cbk": bk, "cbv": bv, "cb1": b1, "cb2": b2}
    cols = []
    for nm in ("wq", "wk", "wv", "w1", "w2"):
        Wn = np.asarray(Ws[nm], np.float32)
        for blk in range(NB):
            cols.append(np.kron(np.eye(8, dtype=np.float32), Wn[blk].T))
    cols.append(np.eye(128, dtype=np.float32))
    cols.append(pat)
    cols.append(rbq)
    for nm in ("cbq", "cbk", "cbv", "cb1", "cb2"):
        bn = np.asarray(Bs[nm], np.float32)
        for blk in range(NB):
            cols.append(np.tile(bn[blk], 8)[:, None])
    fixed = np.concatenate(cols, axis=1)
    ins = []
    for core in range(NCORES):
        sh = x0[core * BS:(core + 1) * BS]                  # [128,S,D]
        xi = sh.reshape(NG, 8, S, D).transpose(1, 3, 0, 2)  # [8,D,NG,S]
        blob = np.concatenate([fixed, xi.reshape(128, NG * S)], axis=1)
        ins.append({"blob": np.ascontiguousarray(blob)})
    return ins


def kernel(**inputs):
    if "nc" not in _CACHE:
        _CACHE["nc"] = _build_program()
    nc = _CACHE["nc"]
    in_maps = _host_prep(**inputs)
    res = run_bass_kernel_spmd(nc, in_maps, core_ids=list(range(NCORES)))
    outs = []
    for core in range(NCORES):
        o = np.asarray(res.results[core]["out"]).reshape(8, D, NG, S)
        outs.append(o.transpose(2, 0, 3, 1).reshape(BS, S, D))  # [128,S,D]
    return np.concatenate(outs, axis=0).astype(np.float32)
